# revision 8
# baseline (speedup 1.0000x reference)
"""Trainium2 Bass kernel for the DRCL loss (nn_DRCL_54004918779968).

Strategy (8 NeuronCores, data-parallel over B*2 half-images):
  - Each core owns half of one image's HW positions (8192 of 16384).
  - Device phase A: z = w1 @ feat (channel-partition layout), one-pass
    bn_stats per channel; 2KB AllReduce combines per-core moments.
  - Device phase B: recompute z in position-partition layout (feat chunks
    stationary), add the folded BN bias C = beta*sd/gamma - mean via K=1
    outer-product matmuls, relu on ScalarE, then mask-stationary matmuls
    reduce the fg/bg masked sums of u = relu(z + C).
  - Host: all index selection (top-k depends only on inputs, not features),
    gathers of the ~160 selected columns per pair via tiny sgemms, and the
    O(KB) contrastive-loss arithmetic, replicating jax fp32 semantics.

Outputs per core: global BN moments [128,4] and raw masked sums [2,256].
"""

import numpy as np

NCORES = 8
B, D, H, W = 4, 256, 128, 128
HW = H * W
HWH = HW // 2          # positions per core
NCH = HWH // 128       # 64 hw chunks of 128
NBLK = 4               # feat DMA blocks of 2048 cols
NT = HWH // 512        # 16 phase-A tiles of 512
NR, NS, TAU, GW = 32, 64, 0.1, 0.5
NEG = np.float32(-1e30)
EPS_BN = 1e-5

_compiled_nc = None
LAST_EXEC_NS = None
TRACE = False


# --------------------------------------------------------------------------
# Device program
# --------------------------------------------------------------------------

def _build_nc():
    import concourse.bacc as bacc
    import concourse.tile as tile
    from concourse import mybir

    AF = mybir.ActivationFunctionType
    dt = mybir.dt.float32

    nc = bacc.Bacc(None, target_bir_lowering=False, num_devices=NCORES)
    feat = nc.dram_tensor("feat", [D, HWH], dt, kind="ExternalInput")
    w1t = nc.dram_tensor("w1t", [128, 2 * D], dt, kind="ExternalInput")
    masks = nc.dram_tensor("masks", [128, NCH * 2], dt, kind="ExternalInput")
    gam = nc.dram_tensor("gam", [128, 2], dt, kind="ExternalInput")
    bet = nc.dram_tensor("bet", [128, 2], dt, kind="ExternalInput")
    ident = nc.dram_tensor("ident", [128, 128], dt, kind="ExternalInput")
    mv_out = nc.dram_tensor("mv_out", [128, 4], dt, kind="ExternalOutput")
    s_out = nc.dram_tensor("s_out", [2, D], dt, kind="ExternalOutput")

    with tile.TileContext(nc) as tc:
        with (
            tc.tile_pool(name="fpool", bufs=1) as fpool,
            tc.tile_pool(name="persist", bufs=1) as persist,
            tc.tile_pool(name="small", bufs=1) as small,
            tc.tile_pool(name="zpsA", bufs=3, space="PSUM") as zpsA,
            tc.tile_pool(name="zpsB", bufs=3, space="PSUM") as zpsB,
            tc.tile_pool(name="sps", bufs=1, space="PSUM") as sps,
            tc.tile_pool(name="upool", bufs=4) as upool,
            tc.tile_pool(name="dram", bufs=2, space="DRAM") as dram,
        ):
            # ---- persistent loads ----
            ws = persist.tile([128, 2, D], dt)   # ws[p, dc, e] = w1[e, dc*128+p]
            nc.sync.dma_start(ws[:], w1t[:].rearrange("p (dc e) -> p dc e", dc=2))
            ms = persist.tile([128, NCH, 2], dt)  # ms[p, c, j]
            nc.sync.dma_start(ms[:], masks[:].rearrange("p (c m) -> p c m", m=2))
            gs = small.tile([128, 2], dt)
            nc.sync.dma_start(gs[:], gam[:])
            bs = small.tile([128, 2], dt)
            nc.sync.dma_start(bs[:], bet[:])
            ids = persist.tile([128, 128], dt)
            nc.sync.dma_start(ids[:], ident[:])
            # preload the sqrt ACT table while phase A runs
            sqwarm = small.tile([1, 1], dt)
            nc.vector.memset(sqwarm[:], 1.0)
            nc.scalar.activation(sqwarm[:], sqwarm[:], AF.Sqrt)

            # feat: fs[p, dc, hw] = feat[dc*128 + p, hw]; 1 MiB DMA blocks
            fs = fpool.tile([128, 2, HWH], dt)
            for blk in range(NBLK):
                cols = slice(blk * 2048, (blk + 1) * 2048)
                for dc in range(2):
                    nc.sync.dma_start(
                        fs[:, dc, cols], feat[dc * 128:(dc + 1) * 128, cols]
                    )

            # phase-B augmented K=2 contraction operands: feat row 255 plus a
            # ones row (x C row) fold the BN bias into the last matmul.
            aug_l = persist.tile([2, HWH], dt)
            nc.gpsimd.memset(aug_l[:], 1.0)  # row 1 stays all-ones
            for blk in range(NBLK):
                cols = slice(blk * 2048, (blk + 1) * 2048)
                nc.sync.dma_start(aug_l[0:1, cols], fs[127:128, 1, cols])
            aug_r = small.tile([2, D], dt)
            nc.sync.dma_start(aug_r[0:1, :], ws[127:128, 1, :])

            # ---- phase A: z = w1 @ feat in [e, hw] layout; bn_stats ----
            stats = persist.tile([128, 2, NT, 6], dt)
            for t in range(NT):
                cols = slice(t * 512, (t + 1) * 512)
                for ec in range(2):
                    zp = zpsA.tile([128, 512], dt)
                    for dc in range(2):
                        nc.tensor.matmul(
                            zp[:],
                            ws[:, dc, ec * 128:(ec + 1) * 128],
                            fs[:, dc, cols],
                            start=(dc == 0),
                            stop=(dc == 1),
                        )
                    nc.vector.bn_stats(stats[:, ec, t, :], zp[:])
            mv = small.tile([128, 2, 2], dt)
            for ec in range(2):
                nc.vector.bn_aggr(mv[:, ec, :], stats[:, ec, :, :])

            # ---- cross-core moment AllReduce ----
            pay = small.tile([128, 4], dt)
            msq = small.tile([128, 2], dt)
            nc.vector.tensor_mul(msq[:], mv[:, :, 0], mv[:, :, 0])
            nc.vector.tensor_copy(pay[:, 0:2], mv[:, :, 0])
            nc.vector.tensor_add(pay[:, 2:4], mv[:, :, 1], msq[:])
            nc.scalar.mul(pay[:], pay[:], 1.0 / NCORES)
            ar_in = dram.tile([128, 4], dt)
            ar_out = dram.tile([128, 4], dt)
            nc.gpsimd.dma_start(ar_in[:], pay[:])
            nc.gpsimd.collective_compute(
                "AllReduce",
                mybir.AluOpType.add,
                replica_groups=[list(range(NCORES))],
                ins=[ar_in.opt()],
                outs=[ar_out.opt()],
            )
            g = small.tile([128, 4], dt)
            nc.gpsimd.dma_start(g[:], ar_out[:])

            # ---- global moments -> sd, C = beta*sd/gamma - mean ----
            gvar = small.tile([128, 2], dt)
            gmsq = small.tile([128, 2], dt)
            nc.vector.tensor_mul(gmsq[:], g[:, 0:2], g[:, 0:2])
            nc.vector.tensor_sub(gvar[:], g[:, 2:4], gmsq[:])
            mvo = small.tile([128, 4], dt)
            nc.vector.tensor_copy(mvo[:, 0:2], g[:, 0:2])
            nc.vector.tensor_copy(mvo[:, 2:4], gvar[:])
            nc.sync.dma_start(mv_out[:], mvo[:])

            veps = small.tile([128, 2], dt)
            nc.vector.tensor_scalar_add(veps[:], gvar[:], EPS_BN)
            sd0 = small.tile([128, 2], dt)
            nc.scalar.activation(sd0[:], veps[:], AF.Sqrt)
            # one Newton step: sd = 0.5*(sd0 + veps/sd0)
            r0 = small.tile([128, 2], dt)
            nc.vector.reciprocal(r0[:], sd0[:])
            t0 = small.tile([128, 2], dt)
            nc.vector.tensor_mul(t0[:], veps[:], r0[:])
            sd = small.tile([128, 2], dt)
            nc.vector.tensor_add(sd[:], sd0[:], t0[:])
            nc.scalar.mul(sd[:], sd[:], 0.5)
            rg = small.tile([128, 2], dt)
            nc.vector.reciprocal(rg[:], gs[:])
            c0 = small.tile([128, 2], dt)
            nc.vector.tensor_mul(c0[:], bs[:], sd[:])
            nc.vector.tensor_mul(c0[:], c0[:], rg[:])
            cc = small.tile([128, 2], dt)
            nc.vector.tensor_sub(cc[:], c0[:], g[:, 0:2])
            # transpose C [128,2] -> [2,128], then place as aug_r row 1 (the
            # [1,256] C row multiplied by the ones row of aug_l)
            cp = sps.tile([2, 128], dt, tag="spshared")
            nc.tensor.transpose(cp[:], cc[:], ids[:])
            cs = small.tile([2, 128], dt)
            nc.scalar.copy(cs[:], cp[:])
            for j in range(2):
                nc.sync.dma_start(aug_r[1:2, j * 128:(j + 1) * 128], cs[j:j + 1, :])

            # ---- phase B+C: u = relu(z + C) in [hw, e] layout; masked sums ----
            sp = sps.tile([2, D], dt, tag="spshared")
            for c in range(NCH):
                cols = slice(c * 128, (c + 1) * 128)
                zp = zpsB.tile([128, D], dt)
                # d rows 0..127 (K=128), 128..254 (K=127), then {row 255, ones
                # x C} as a K=2 augmented matmul
                nc.tensor.matmul(
                    zp[:], fs[:, 0, cols], ws[:, 0, :],
                    start=True, stop=False, skip_group_check=True,
                )
                nc.tensor.matmul(
                    zp[:], fs[0:127, 1, cols], ws[0:127, 1, :],
                    start=False, stop=False, skip_group_check=True,
                )
                nc.tensor.matmul(
                    zp[:], aug_l[:, cols], aug_r[:],
                    start=False, stop=True, skip_group_check=True,
                )
                u = upool.tile([128, D], dt)
                nc.scalar.activation(u[:], zp[:], AF.Relu)
                nc.tensor.matmul(
                    sp[:],
                    ms[:, c, :],
                    u[:],
                    start=(c == 0),
                    stop=(c == NCH - 1),
                )
            so = small.tile([2, D], dt)
            nc.scalar.copy(so[:], sp[:])
            nc.sync.dma_start(s_out[:], so[:])

    nc.compile()
    return nc


def _get_nc():
    global _compiled_nc
    if _compiled_nc is None:
        _compiled_nc = _build_nc()
    return _compiled_nc


# --------------------------------------------------------------------------
# Host orchestration
# --------------------------------------------------------------------------

def _masks_from_inputs(labels, prob_ori, prob_aug, unc):
    rel = prob_ori.argmax(1) == prob_aug.argmax(1)          # [B,H,W]
    diff = unc > 0.5
    valid = (rel & diff).reshape(B, -1)
    lab = labels.reshape(B, -1)
    m1 = valid & (lab == 1)
    m0 = valid & (lab == 0)
    return m1, m0


def _run_device(feat, w1, gamma, beta, m1, m0):
    global LAST_EXEC_NS
    from concourse.bass_utils import run_bass_kernel_spmd

    f32 = np.float32
    nc = _get_nc()
    w1t_p = np.ascontiguousarray(
        w1.T.reshape(2, 128, D).transpose(1, 0, 2).reshape(128, 2 * D)
    ).astype(f32)
    gam_p = np.ascontiguousarray(gamma.reshape(2, 128).T).astype(f32)
    bet_p = np.ascontiguousarray(beta.reshape(2, 128).T).astype(f32)
    ident = np.eye(128, dtype=f32)
    in_maps = []
    for c in range(NCORES):
        b, hhalf = c // 2, c % 2
        cols = slice(hhalf * HWH, (hhalf + 1) * HWH)
        fh = np.ascontiguousarray(feat[b].reshape(D, HW)[:, cols]).astype(f32)
        mm = np.stack([m1[b, cols], m0[b, cols]], axis=1).astype(f32)  # [HWH,2]
        mp = np.ascontiguousarray(
            mm.reshape(NCH, 128, 2).transpose(1, 0, 2).reshape(128, NCH * 2)
        )
        in_maps.append(
            {"feat": fh, "w1t": w1t_p, "masks": mp, "gam": gam_p, "bet": bet_p,
             "ident": ident}
        )
    res = run_bass_kernel_spmd(
        nc, in_maps, core_ids=list(range(NCORES)), trace=TRACE
    )
    if TRACE:
        LAST_EXEC_NS = res.exec_time_ns
    mv = res.results[0]["mv_out"]
    gmean = np.concatenate([mv[:, 0], mv[:, 1]]).astype(f32)
    gvar = np.concatenate([mv[:, 2], mv[:, 3]]).astype(f32)
    s_raw = [res.results[c]["s_out"].astype(f32) for c in range(NCORES)]
    return gmean, gvar, s_raw


def _topk(vals, k):
    return np.argsort(-vals, kind="stable")[:k]


def _nrm_rows(x):
    n = np.linalg.norm(x, axis=-1, keepdims=True)
    return x / np.maximum(n, np.float32(1e-12))


def _host_finish(inputs, gmean, gvar, s_raw, m1, m0):
    f32 = np.float32
    feat = inputs["feat"]; unc = inputs["unc"]
    r_anc = inputs["r_anc"]; r_pos = inputs["r_pos"]; r_neg = inputs["r_neg"]
    w1 = inputs["w1"]; b1 = inputs["b1"]
    gamma = inputs["gamma"]; beta = inputs["beta"]
    w2 = inputs["w2"]; b2 = inputs["b2"]

    uf = unc.reshape(B, -1)
    sd = np.sqrt(gvar + f32(EPS_BN)).astype(f32)
    A = (gamma / sd).astype(f32)

    # ---- local loss ----
    bl = np.zeros((B, 2), f32)
    inc = np.zeros((B, 2), bool)
    for b in range(B):
        featb = feat[b].reshape(D, HW)

        def proj_cols(idx):
            z = (w1 @ featb[:, idx]).astype(f32) + b1[:, None]
            # BN uses stats of x = z + b1: x - mu_x = z - gmean (b1 cancels);
            # gmean here excludes b1, so subtract (gmean + b1) from x.
            xc = z - (gmean + b1)[:, None]
            y = np.maximum(A[:, None] * xc + beta[:, None], f32(0.0)).astype(f32)
            return (w2 @ y + b2[:, None]).astype(f32)  # [D, n]

        for cl in range(2):
            am = m1[b] if cl == 0 else m0[b]
            nm = m0[b] if cl == 0 else m1[b]
            ra, rp, rn = r_anc[b, cl], r_pos[b, cl], r_neg[b, cl]

            def sel(mask, r, k):
                idx = _topk(np.where(mask, r, NEG).astype(f32), k)
                return idx, mask[idx]

            def hard(mask, r):
                cidx, cval = sel(mask, r, 2 * NS)
                t = _topk(np.where(cval, uf[b][cidx], NEG).astype(f32), NS)
                return cidx[t], cval[t]

            aidx, aval = sel(am, ra, NR)
            pidx, pval = hard(am, rp)
            nidx, nval = hard(nm, rn)
            q = _nrm_rows(proj_cols(aidx).T)
            P = _nrm_rows(proj_cols(pidx).T)
            Ng = _nrm_rows(proj_cols(nidx).T)
            pw = pval.astype(f32)[:, None]
            nw = nval.astype(f32)[:, None]
            p = (np.exp((P @ q.T).astype(f32) / f32(TAU)) * pw).sum(0).astype(f32)
            n_ = (np.exp((Ng @ q.T).astype(f32) / f32(TAU)) * nw).sum(0).astype(f32)
            inc_ = bool(am.sum() >= 1) and bool(nm.sum() >= 1)
            p = p + f32(1.0) - f32(inc_)
            per = (-np.log(p / (p + n_ + f32(1e-8)))).astype(f32)
            af = aval.astype(f32)
            blv = f32((per * af).sum()) / np.maximum(f32(af.sum()), f32(1.0))
            bl[b, cl] = blv if inc_ else f32(0.0)
            inc[b, cl] = inc_
    l_local = f32(bl.sum()) / f32(max(int(inc.sum()), 1))

    # ---- global loss ----
    fgf = m1.astype(f32); bgf = m0.astype(f32)
    cf = fgf.sum(1); cb = bgf.sum(1)
    m_fg = np.zeros((B, D), f32)
    m_bg = np.zeros((B, D), f32)
    for b in range(B):
        s = s_raw[2 * b] + s_raw[2 * b + 1]       # [2, D] raw sums of u
        s_y_fg = (A * s[0]).astype(f32)
        s_y_bg = (A * s[1]).astype(f32)
        m_fg[b] = (w2 @ s_y_fg + b2 * cf[b]) / np.maximum(cf[b], f32(1.0))
        m_bg[b] = (w2 @ s_y_bg + b2 * cb[b]) / np.maximum(cb[b], f32(1.0))
    vg = (cf >= 1) & (cb >= 1)
    qf = _nrm_rows(m_fg); qb = _nrm_rows(m_bg)
    Mm = (
        (np.arange(B)[None, :] <= np.arange(B)[:, None]) & vg[None, :]
    ).astype(f32)
    Sf = np.exp((qb @ qf.T).astype(f32) / f32(TAU))
    Sb = np.exp((qf @ qb.T).astype(f32) / f32(TAU))
    nf = np.einsum("jb,bj->b", Sf, Mm).astype(f32)
    nb = np.einsum("jb,bj->b", Sb, Mm).astype(f32)
    pf = np.exp((qf * qf).sum(-1) / f32(TAU)).astype(f32)
    pb = np.exp((qb * qb).sum(-1) / f32(TAU)).astype(f32)
    lg = -np.log(pf / (pf + nf + f32(1e-8))) - np.log(pb / (pb + nb + f32(1e-8)))
    l_global = f32((vg.astype(f32) * lg).sum()) / f32(max(int(vg.sum()), 1))

    total = f32(l_local + f32(GW) * l_global)
    return total, f32(l_local), f32(l_global)


def kernel(**inputs):
    inputs = {k: np.asarray(v) for k, v in inputs.items()}
    m1, m0 = _masks_from_inputs(
        inputs["labels"], inputs["prob_ori"], inputs["prob_aug"], inputs["unc"]
    )
    gmean, gvar, s_raw = _run_device(
        inputs["feat"], inputs["w1"], inputs["gamma"], inputs["beta"], m1, m0
    )
    return _host_finish(inputs, gmean, gvar, s_raw, m1, m0)


# revision 14
# speedup vs baseline: 1.7989x; 1.7989x over previous
"""Trainium2 Bass kernel for the DRCL loss (nn_DRCL_54004918779968).

Strategy (8 NeuronCores, data-parallel over B*2 half-images):
  - Each core owns half of one image's HW positions (8192 of 16384).
  - Device phase A: z = w1 @ feat (channel-partition layout), one-pass
    bn_stats per channel; 2KB AllReduce combines per-core moments.
  - Device phase B: recompute z in position-partition layout (feat chunks
    stationary), add the folded BN bias C = beta*sd/gamma - mean via K=1
    outer-product matmuls, relu on ScalarE, then mask-stationary matmuls
    reduce the fg/bg masked sums of u = relu(z + C).
  - Host: all index selection (top-k depends only on inputs, not features),
    gathers of the ~160 selected columns per pair via tiny sgemms, and the
    O(KB) contrastive-loss arithmetic, replicating jax fp32 semantics.

Outputs per core: global BN moments [128,4] and raw masked sums [2,256].
"""

import numpy as np

NCORES = 8
B, D, H, W = 4, 256, 128, 128
HW = H * W
HWH = HW // 2          # positions per core
NCH = HWH // 128       # 64 hw chunks of 128
NBLK = 4               # feat DMA blocks of 2048 cols
NT = HWH // 512        # 16 phase-A tiles of 512
NR, NS, TAU, GW = 32, 64, 0.1, 0.5
NEG = np.float32(-1e30)
EPS_BN = 1e-5

_compiled_nc = None
LAST_EXEC_NS = None
TRACE = False


# --------------------------------------------------------------------------
# Device program
# --------------------------------------------------------------------------

def _build_nc():
    import concourse.bacc as bacc
    import concourse.tile as tile
    from concourse import mybir

    AF = mybir.ActivationFunctionType
    dt = mybir.dt.float32
    bt = mybir.dt.bfloat16

    nc = bacc.Bacc(None, target_bir_lowering=False, num_devices=NCORES)
    feat = nc.dram_tensor("feat", [D, HWH], bt, kind="ExternalInput")
    w1t = nc.dram_tensor("w1t", [128, 2 * D], bt, kind="ExternalInput")
    masks = nc.dram_tensor("masks", [128, NCH * 2], bt, kind="ExternalInput")
    gam = nc.dram_tensor("gam", [128, 2], dt, kind="ExternalInput")
    bet = nc.dram_tensor("bet", [128, 2], dt, kind="ExternalInput")
    ident = nc.dram_tensor("ident", [128, 128], dt, kind="ExternalInput")
    mv_out = nc.dram_tensor("mv_out", [128, 4], dt, kind="ExternalOutput")
    s_out = nc.dram_tensor("s_out", [2, D], dt, kind="ExternalOutput")

    with tile.TileContext(nc) as tc:
        with (
            tc.tile_pool(name="fpool", bufs=1) as fpool,
            tc.tile_pool(name="persist", bufs=1) as persist,
            tc.tile_pool(name="small", bufs=1) as small,
            tc.tile_pool(name="zpsA", bufs=3, space="PSUM") as zpsA,
            tc.tile_pool(name="zpsB", bufs=3, space="PSUM") as zpsB,
            tc.tile_pool(name="sps", bufs=1, space="PSUM") as sps,
            tc.tile_pool(name="upool", bufs=4) as upool,
            tc.tile_pool(name="dram", bufs=2, space="DRAM") as dram,
        ):
            # ---- persistent loads ----
            ws = persist.tile([128, 2, D], bt)   # ws[p, dc, e] = w1[e, dc*128+p]
            nc.sync.dma_start(ws[:], w1t[:].rearrange("p (dc e) -> p dc e", dc=2))
            ms = persist.tile([128, NCH, 2], bt)  # ms[p, c, j]
            nc.sync.dma_start(ms[:], masks[:].rearrange("p (c m) -> p c m", m=2))
            gs = small.tile([128, 2], dt)
            nc.sync.dma_start(gs[:], gam[:])
            bs = small.tile([128, 2], dt)
            nc.sync.dma_start(bs[:], bet[:])
            ids = persist.tile([128, 128], dt)
            nc.sync.dma_start(ids[:], ident[:])
            # preload the sqrt ACT table while phase A runs
            sqwarm = small.tile([1, 1], dt)
            nc.vector.memset(sqwarm[:], 1.0)
            nc.scalar.activation(sqwarm[:], sqwarm[:], AF.Sqrt)

            # feat: fs[p, dc, hw] = feat[dc*128 + p, hw]; 0.5 MiB DMA blocks
            fs = fpool.tile([128, 2, HWH], bt)
            for blk in range(NBLK):
                cols = slice(blk * 2048, (blk + 1) * 2048)
                for dc in range(2):
                    nc.sync.dma_start(
                        fs[:, dc, cols], feat[dc * 128:(dc + 1) * 128, cols]
                    )

            # phase-B augmented K=3 contraction operands: feat row 255 plus
            # two ones rows (x C split into bf16 hi + lo) folding the BN bias
            # into the last matmul without bf16 rounding of C.
            aug_l = persist.tile([3, HWH], bt)
            nc.gpsimd.memset(aug_l[:], 1.0)  # rows 1,2 stay all-ones
            for blk in range(NBLK):
                cols = slice(blk * 2048, (blk + 1) * 2048)
                nc.sync.dma_start(aug_l[0:1, cols], fs[127:128, 1, cols])
            aug_r = small.tile([3, D], bt)
            nc.sync.dma_start(aug_r[0:1, :], ws[127:128, 1, :])

            # ---- phase A: z = w1 @ feat in [e, hw] layout; bn_stats ----
            stats = persist.tile([128, 2, NT, 6], dt)
            for t in range(NT):
                cols = slice(t * 512, (t + 1) * 512)
                for ec in range(2):
                    zp = zpsA.tile([128, 512], dt)
                    for dc in range(2):
                        nc.tensor.matmul(
                            zp[:],
                            ws[:, dc, ec * 128:(ec + 1) * 128],
                            fs[:, dc, cols],
                            start=(dc == 0),
                            stop=(dc == 1),
                        )
                    nc.vector.bn_stats(stats[:, ec, t, :], zp[:])
            mv = small.tile([128, 2, 2], dt)
            for ec in range(2):
                nc.vector.bn_aggr(mv[:, ec, :], stats[:, ec, :, :])

            # ---- cross-core moment AllReduce ----
            pay = small.tile([128, 4], dt)
            msq = small.tile([128, 2], dt)
            nc.vector.tensor_mul(msq[:], mv[:, :, 0], mv[:, :, 0])
            nc.vector.tensor_copy(pay[:, 0:2], mv[:, :, 0])
            nc.vector.tensor_add(pay[:, 2:4], mv[:, :, 1], msq[:])
            nc.scalar.mul(pay[:], pay[:], 1.0 / NCORES)
            ar_in = dram.tile([128, 4], dt)
            ar_out = dram.tile([128, 4], dt)
            nc.gpsimd.dma_start(ar_in[:], pay[:])
            nc.gpsimd.collective_compute(
                "AllReduce",
                mybir.AluOpType.add,
                replica_groups=[list(range(NCORES))],
                ins=[ar_in.opt()],
                outs=[ar_out.opt()],
            )
            g = small.tile([128, 4], dt)
            nc.gpsimd.dma_start(g[:], ar_out[:])

            # ---- global moments -> sd, C = beta*sd/gamma - mean ----
            gvar = small.tile([128, 2], dt)
            gmsq = small.tile([128, 2], dt)
            nc.vector.tensor_mul(gmsq[:], g[:, 0:2], g[:, 0:2])
            nc.vector.tensor_sub(gvar[:], g[:, 2:4], gmsq[:])
            mvo = small.tile([128, 4], dt)
            nc.vector.tensor_copy(mvo[:, 0:2], g[:, 0:2])
            nc.vector.tensor_copy(mvo[:, 2:4], gvar[:])
            nc.sync.dma_start(mv_out[:], mvo[:])

            veps = small.tile([128, 2], dt)
            nc.vector.tensor_scalar_add(veps[:], gvar[:], EPS_BN)
            sd0 = small.tile([128, 2], dt)
            nc.scalar.activation(sd0[:], veps[:], AF.Sqrt)
            # one Newton step: sd = 0.5*(sd0 + veps/sd0)
            r0 = small.tile([128, 2], dt)
            nc.vector.reciprocal(r0[:], sd0[:])
            t0 = small.tile([128, 2], dt)
            nc.vector.tensor_mul(t0[:], veps[:], r0[:])
            sd = small.tile([128, 2], dt)
            nc.vector.tensor_add(sd[:], sd0[:], t0[:])
            nc.scalar.mul(sd[:], sd[:], 0.5)
            rg = small.tile([128, 2], dt)
            nc.vector.reciprocal(rg[:], gs[:])
            c0 = small.tile([128, 2], dt)
            nc.vector.tensor_mul(c0[:], bs[:], sd[:])
            nc.vector.tensor_mul(c0[:], c0[:], rg[:])
            cc = small.tile([128, 2], dt)
            nc.vector.tensor_sub(cc[:], c0[:], g[:, 0:2])
            # transpose C [128,2] -> [2,128]; split into bf16 hi + residual lo
            # and place as aug_r rows 1,2 (multiplied by the ones rows)
            cp = sps.tile([2, 128], dt, tag="spshared")
            nc.tensor.transpose(cp[:], cc[:], ids[:])
            cs = small.tile([2, 128], dt)
            nc.scalar.copy(cs[:], cp[:])
            cs_hi = small.tile([2, 128], bt)
            nc.vector.tensor_copy(cs_hi[:], cs[:])
            cs_lo = small.tile([2, 128], dt)
            nc.vector.tensor_sub(cs_lo[:], cs[:], cs_hi[:])
            cs_lo_b = small.tile([2, 128], bt)
            nc.vector.tensor_copy(cs_lo_b[:], cs_lo[:])
            for j in range(2):
                nc.sync.dma_start(
                    aug_r[1:2, j * 128:(j + 1) * 128], cs_hi[j:j + 1, :]
                )
                nc.sync.dma_start(
                    aug_r[2:3, j * 128:(j + 1) * 128], cs_lo_b[j:j + 1, :]
                )

            # ---- phase B+C: u = relu(z + C) in [hw, e] layout; masked sums ----
            sp = sps.tile([2, D], dt, tag="spshared")
            for c in range(NCH):
                cols = slice(c * 128, (c + 1) * 128)
                zp = zpsB.tile([128, D], dt)
                # d rows 0..127 (K=128), 128..254 (K=127), then {row 255, ones
                # x C} as a K=2 augmented matmul
                nc.tensor.matmul(
                    zp[:], fs[:, 0, cols], ws[:, 0, :],
                    start=True, stop=False, skip_group_check=True,
                )
                nc.tensor.matmul(
                    zp[:], fs[0:127, 1, cols], ws[0:127, 1, :],
                    start=False, stop=False, skip_group_check=True,
                )
                nc.tensor.matmul(
                    zp[:], aug_l[:, cols], aug_r[:],
                    start=False, stop=True, skip_group_check=True,
                )
                u = upool.tile([128, D], bt)
                nc.scalar.activation(u[:], zp[:], AF.Relu)
                nc.tensor.matmul(
                    sp[:],
                    ms[:, c, :],
                    u[:],
                    start=(c == 0),
                    stop=(c == NCH - 1),
                )
            so = small.tile([2, D], dt)
            nc.scalar.copy(so[:], sp[:])
            nc.sync.dma_start(s_out[:], so[:])

    nc.compile()
    return nc


def _get_nc():
    global _compiled_nc
    if _compiled_nc is None:
        _compiled_nc = _build_nc()
    return _compiled_nc


# --------------------------------------------------------------------------
# Host orchestration
# --------------------------------------------------------------------------

def _masks_from_inputs(labels, prob_ori, prob_aug, unc):
    rel = prob_ori.argmax(1) == prob_aug.argmax(1)          # [B,H,W]
    diff = unc > 0.5
    valid = (rel & diff).reshape(B, -1)
    lab = labels.reshape(B, -1)
    m1 = valid & (lab == 1)
    m0 = valid & (lab == 0)
    return m1, m0


def _run_device(feat, w1, gamma, beta, m1, m0):
    global LAST_EXEC_NS
    import ml_dtypes
    from concourse.bass_utils import run_bass_kernel_spmd

    f32 = np.float32
    bf16 = ml_dtypes.bfloat16
    nc = _get_nc()
    w1t_p = np.ascontiguousarray(
        w1.T.reshape(2, 128, D).transpose(1, 0, 2).reshape(128, 2 * D)
    ).astype(bf16)
    gam_p = np.ascontiguousarray(gamma.reshape(2, 128).T).astype(f32)
    bet_p = np.ascontiguousarray(beta.reshape(2, 128).T).astype(f32)
    ident = np.eye(128, dtype=f32)
    in_maps = []
    for c in range(NCORES):
        b, hhalf = c // 2, c % 2
        cols = slice(hhalf * HWH, (hhalf + 1) * HWH)
        fh = np.ascontiguousarray(feat[b].reshape(D, HW)[:, cols]).astype(bf16)
        mm = np.stack([m1[b, cols], m0[b, cols]], axis=1).astype(bf16)  # [HWH,2]
        mp = np.ascontiguousarray(
            mm.reshape(NCH, 128, 2).transpose(1, 0, 2).reshape(128, NCH * 2)
        )
        in_maps.append(
            {"feat": fh, "w1t": w1t_p, "masks": mp, "gam": gam_p, "bet": bet_p,
             "ident": ident}
        )
    res = run_bass_kernel_spmd(
        nc, in_maps, core_ids=list(range(NCORES)), trace=TRACE
    )
    if TRACE:
        LAST_EXEC_NS = res.exec_time_ns
    mv = res.results[0]["mv_out"]
    gmean = np.concatenate([mv[:, 0], mv[:, 1]]).astype(f32)
    gvar = np.concatenate([mv[:, 2], mv[:, 3]]).astype(f32)
    s_raw = [res.results[c]["s_out"].astype(f32) for c in range(NCORES)]
    return gmean, gvar, s_raw


def _topk(vals, k):
    return np.argsort(-vals, kind="stable")[:k]


def _nrm_rows(x):
    n = np.linalg.norm(x, axis=-1, keepdims=True)
    return x / np.maximum(n, np.float32(1e-12))


def _host_finish(inputs, gmean, gvar, s_raw, m1, m0):
    f32 = np.float32
    feat = inputs["feat"]; unc = inputs["unc"]
    r_anc = inputs["r_anc"]; r_pos = inputs["r_pos"]; r_neg = inputs["r_neg"]
    w1 = inputs["w1"]; b1 = inputs["b1"]
    gamma = inputs["gamma"]; beta = inputs["beta"]
    w2 = inputs["w2"]; b2 = inputs["b2"]

    uf = unc.reshape(B, -1)
    sd = np.sqrt(gvar + f32(EPS_BN)).astype(f32)
    A = (gamma / sd).astype(f32)

    # ---- local loss ----
    bl = np.zeros((B, 2), f32)
    inc = np.zeros((B, 2), bool)
    for b in range(B):
        featb = feat[b].reshape(D, HW)

        def proj_cols(idx):
            z = (w1 @ featb[:, idx]).astype(f32) + b1[:, None]
            # BN uses stats of x = z + b1: x - mu_x = z - gmean (b1 cancels);
            # gmean here excludes b1, so subtract (gmean + b1) from x.
            xc = z - (gmean + b1)[:, None]
            y = np.maximum(A[:, None] * xc + beta[:, None], f32(0.0)).astype(f32)
            return (w2 @ y + b2[:, None]).astype(f32)  # [D, n]

        for cl in range(2):
            am = m1[b] if cl == 0 else m0[b]
            nm = m0[b] if cl == 0 else m1[b]
            ra, rp, rn = r_anc[b, cl], r_pos[b, cl], r_neg[b, cl]

            def sel(mask, r, k):
                idx = _topk(np.where(mask, r, NEG).astype(f32), k)
                return idx, mask[idx]

            def hard(mask, r):
                cidx, cval = sel(mask, r, 2 * NS)
                t = _topk(np.where(cval, uf[b][cidx], NEG).astype(f32), NS)
                return cidx[t], cval[t]

            aidx, aval = sel(am, ra, NR)
            pidx, pval = hard(am, rp)
            nidx, nval = hard(nm, rn)
            q = _nrm_rows(proj_cols(aidx).T)
            P = _nrm_rows(proj_cols(pidx).T)
            Ng = _nrm_rows(proj_cols(nidx).T)
            pw = pval.astype(f32)[:, None]
            nw = nval.astype(f32)[:, None]
            p = (np.exp((P @ q.T).astype(f32) / f32(TAU)) * pw).sum(0).astype(f32)
            n_ = (np.exp((Ng @ q.T).astype(f32) / f32(TAU)) * nw).sum(0).astype(f32)
            inc_ = bool(am.sum() >= 1) and bool(nm.sum() >= 1)
            p = p + f32(1.0) - f32(inc_)
            per = (-np.log(p / (p + n_ + f32(1e-8)))).astype(f32)
            af = aval.astype(f32)
            blv = f32((per * af).sum()) / np.maximum(f32(af.sum()), f32(1.0))
            bl[b, cl] = blv if inc_ else f32(0.0)
            inc[b, cl] = inc_
    l_local = f32(bl.sum()) / f32(max(int(inc.sum()), 1))

    # ---- global loss ----
    fgf = m1.astype(f32); bgf = m0.astype(f32)
    cf = fgf.sum(1); cb = bgf.sum(1)
    m_fg = np.zeros((B, D), f32)
    m_bg = np.zeros((B, D), f32)
    for b in range(B):
        s = s_raw[2 * b] + s_raw[2 * b + 1]       # [2, D] raw sums of u
        s_y_fg = (A * s[0]).astype(f32)
        s_y_bg = (A * s[1]).astype(f32)
        m_fg[b] = (w2 @ s_y_fg + b2 * cf[b]) / np.maximum(cf[b], f32(1.0))
        m_bg[b] = (w2 @ s_y_bg + b2 * cb[b]) / np.maximum(cb[b], f32(1.0))
    vg = (cf >= 1) & (cb >= 1)
    qf = _nrm_rows(m_fg); qb = _nrm_rows(m_bg)
    Mm = (
        (np.arange(B)[None, :] <= np.arange(B)[:, None]) & vg[None, :]
    ).astype(f32)
    Sf = np.exp((qb @ qf.T).astype(f32) / f32(TAU))
    Sb = np.exp((qf @ qb.T).astype(f32) / f32(TAU))
    nf = np.einsum("jb,bj->b", Sf, Mm).astype(f32)
    nb = np.einsum("jb,bj->b", Sb, Mm).astype(f32)
    pf = np.exp((qf * qf).sum(-1) / f32(TAU)).astype(f32)
    pb = np.exp((qb * qb).sum(-1) / f32(TAU)).astype(f32)
    lg = -np.log(pf / (pf + nf + f32(1e-8))) - np.log(pb / (pb + nb + f32(1e-8)))
    l_global = f32((vg.astype(f32) * lg).sum()) / f32(max(int(vg.sum()), 1))

    total = f32(l_local + f32(GW) * l_global)
    return total, f32(l_local), f32(l_global)


def kernel(**inputs):
    inputs = {k: np.asarray(v) for k, v in inputs.items()}
    m1, m0 = _masks_from_inputs(
        inputs["labels"], inputs["prob_ori"], inputs["prob_aug"], inputs["unc"]
    )
    gmean, gvar, s_raw = _run_device(
        inputs["feat"], inputs["w1"], inputs["gamma"], inputs["beta"], m1, m0
    )
    return _host_finish(inputs, gmean, gvar, s_raw, m1, m0)
